# revision 1
# baseline (speedup 1.0000x reference)
"""LIF spike kernel (T=4 scan with threshold reset) on 8 TRN2 NeuronCores.

Recurrence per element (tau=1, thresh=1):
    s_t     = m_{t-1} + x_t
    spike_t = (s_t > 1)           -> output
    m_t     = s_t * (s_t <= 1)    -> threshold reset

Sharding: pure data-parallel over the batch axis (dim 1, 64 -> 8 per core).

Final design (HW exec ~74 us/core vs ~110 us for the f32-out all-DVE
baseline):
  - spikes leave the device as int8 sign planes: q_t = Sign(1 - s_t) in
    {-1,0,1} on the otherwise-idle Scalar (ACT) engine (exact at the
    threshold: Sign is not LUT-interpolated, and s==1 maps to q==0 -> no
    spike, matching the strict >). Host maps q==-1 -> 1.0f for free.
    Output traffic drops 4x (16 MiB -> 4 MiB per core).
  - DVE runs only the serial recurrence: per chunk 3 stt resets
    (m = s*(s<=1), one fused instruction each) + 3 tensor_tensor adds,
    with s_t computed in place over the per-plane x tiles so ACT reads
    never block the DVE chain. This is the hard wall: 6 fp32 two-tensor
    ops/element-row = ~56 us of DVE at 128 lanes x 0.96 GHz.
    (Measured dead ends: GPSIMD tensor ops steal a DVE SBUF port and
    inflate DVE ops ~30%; CCE accum-DMA adds run at half wire rate and
    starve the loads; TensorE matmul-adds need SBUF sources + PSUM dest,
    which re-triggers the stt both-PSUM-source restriction.)
  - per-plane HWDGE loads on the sync ring (DVE's first op waits on one
    1 MB plane, not a whole chunk; the first plane is further split in
    halves); int8 stores ride the scalar ring right after each sign.
  - asymmetric chunk widths: big first chunks leave the load stream a few
    us of headroom over the rate-matched DVE (absorbing the stores'
    wire-steal); a small last chunk keeps the tail short. bufs=4 so tile
    slot recycling (released only after ACT's sign) never throttles loads.
"""

import numpy as np

import concourse.bacc as bacc
import concourse.mybir as mybir
import concourse.tile as tile
from concourse import bass_utils

T = 4
B_FULL = 64
C, H, W = 128, 32, 32
N_CORES = 8
B_LOC = B_FULL // N_CORES            # 8
N = B_LOC * C * H * W                # 1048576 elements per core per timestep
P = 128                              # SBUF partitions
NP = N // P                          # 8192 elements per partition row

_LE = mybir.AluOpType.is_le
_MUL = mybir.AluOpType.mult
_ADD = mybir.AluOpType.add
_F32 = mybir.dt.float32
_I8 = mybir.dt.int8
_SIGN = mybir.ActivationFunctionType.Sign

# chunk widths (elements per partition); sum must be NP. Big first chunk =
# natural load headroom (loads and DVE are rate-matched, so the pipeline
# needs a few us of buffered input to absorb the stores' wire-steal);
# tiny last chunk = short tail.
FS = [2048, 2048, 2048, 1536, 512]
assert sum(FS) == NP

_nc_cache = None


def _build(fs=tuple(FS), bufs=4):
    nc = bacc.Bacc(
        "TRN2",
        target_bir_lowering=False,
        debug=False,
        enable_asserts=False,
    )
    x_d = nc.dram_tensor("x", [T, N], _F32, kind="ExternalInput").ap()
    y_d = nc.dram_tensor("y", [T, N], _I8, kind="ExternalOutput").ap()

    def xsl(t, base, f):
        return x_d[t, P * base : P * (base + f)].rearrange("(p f) -> p f", p=P)

    def ysl(t, base, f):
        return y_d[t, P * base : P * (base + f)].rearrange("(p f) -> p f", p=P)

    nchunk = len(fs)
    with tile.TileContext(nc) as tc:
        with (
            tc.tile_pool(name="xx", bufs=bufs) as xp,
            tc.tile_pool(name="mm", bufs=3) as mp,
            tc.tile_pool(name="qq", bufs=bufs) as qp,
        ):
            base = 0
            for j, F in enumerate(fs):
                last = j == nchunk - 1
                sl = []
                for t in range(T):
                    xt = xp.tile([P, F], _F32, tag=f"x{t}", name=f"x{t}_{j}")
                    if j == 0 and t <= 1:
                        # split the first loads so DVE starts after half a
                        # plane and the t=1 add isn't gated on a full plane
                        h = F // 2
                        src = xsl(t, base, F)
                        nc.sync.dma_start(xt[:, :h], src[:, :h])
                        nc.sync.dma_start(xt[:, h:], src[:, h:])
                    else:
                        nc.sync.dma_start(xt[:], xsl(t, base, F))
                    sl.append(xt[:])
                m = mp.tile([P, F], _F32, tag="m", name=f"m_{j}")
                ydst = [ysl(t, base, F) for t in range(T)]

                v = nc.vector

                def sign_store(src, t, off=0, w=F, k=""):
                    q = qp.tile([P, w], _I8, tag=f"q{t}{k}", name=f"q{t}{k}_{j}")
                    nc.scalar.activation(q[:], src, _SIGN, bias=1.0, scale=-1.0)
                    nc.scalar.dma_start(ydst[t][:, off : off + w], q[:])

                # t = 0
                if j == 0:
                    h = F // 2
                    for off, k in ((0, "a"), (h, "b")):
                        s = sl[0][:, off : off + h]
                        v.scalar_tensor_tensor(
                            m[:, off : off + h], s, 1.0, s, _LE, _MUL
                        )
                        sign_store(s, 0, off, h, k)
                else:
                    v.scalar_tensor_tensor(m[:], sl[0], 1.0, sl[0], _LE, _MUL)
                    sign_store(sl[0], 0)
                # t = 1, 2
                for t in (1, 2):
                    if j == 0 and t == 1:
                        h = F // 2
                        for off, k in ((0, "a"), (h, "b")):
                            s = sl[1][:, off : off + h]
                            mh = m[:, off : off + h]
                            v.tensor_tensor(s, mh, s, _ADD)
                            v.scalar_tensor_tensor(mh, s, 1.0, s, _LE, _MUL)
                            sign_store(s, 1, off, h, k)
                    else:
                        v.tensor_tensor(sl[t], m[:], sl[t], _ADD)
                        v.scalar_tensor_tensor(m[:], sl[t], 1.0, sl[t], _LE, _MUL)
                        sign_store(sl[t], t)
                # t = 3 (terminal add, no reset)
                v.tensor_tensor(sl[3], m[:], sl[3], _ADD)
                sign_store(sl[3], 3)
                base += F

    nc.compile()
    return nc


def _get_nc():
    global _nc_cache
    if _nc_cache is None:
        _nc_cache = _build()
    return _nc_cache


def _run(x, **spmd_kwargs):
    x = np.asarray(x, dtype=np.float32)
    assert x.shape == (T, B_FULL, C, H, W), x.shape
    in_maps = [
        {
            "x": np.ascontiguousarray(
                x[:, c * B_LOC : (c + 1) * B_LOC]
            ).reshape(T, N)
        }
        for c in range(N_CORES)
    ]
    res = bass_utils.run_bass_kernel_spmd(
        _get_nc(), in_maps, core_ids=list(range(N_CORES)), **spmd_kwargs
    )
    out = np.empty((T, B_FULL, C, H, W), dtype=np.float32)
    for c in range(N_CORES):
        y = res.results[c]["y"]
        sp = (y.reshape(T, N) == -1).astype(np.float32)
        out[:, c * B_LOC : (c + 1) * B_LOC] = sp.reshape(T, B_LOC, C, H, W)
    return out, res


def kernel(x):
    out, _ = _run(x)
    return out



# revision 2
# speedup vs baseline: 1.1161x; 1.1161x over previous
"""LIF spike kernel (T=4 scan with threshold reset) on 8 TRN2 NeuronCores.

Recurrence per element (tau=1, thresh=1):
    s_t     = m_{t-1} + x_t
    spike_t = (s_t > 1)           -> output
    m_t     = s_t * (s_t <= 1)    -> threshold reset

Sharding: pure data-parallel over the batch axis (dim 1, 64 -> 8 per core).

Design (v2, custom-DVE):
  - Carry s_t (pre-reset membrane) instead of m_t: s_{t+1} = s_t*(s_t<=1)
    + x_{t+1} is ONE fused custom DVE instruction (LIF_STEP_ANT), so the
    serial recurrence costs 3 DVE passes instead of 6 (fp32 tensor ops
    run at 1 elem/lane/cycle regardless of fusion). Bit-exact: the mult
    is by an exact 0/1 mask and the add is a single IEEE fp32 add, same
    order as the reference.
  - Spikes leave the device packed 2-per-byte: y = (s_a>1)*2 + (s_b>1)
    (PACK_SPIKES2_ANT, int8 out straight from the DVE write port, values
    0..3 exact). Output wire traffic is 2 MiB/core vs 4 MiB for the int8
    sign planes and 16 MiB for f32. (s-1>0 <=> s>1 in fp32: Sterbenz on
    (1,2), sign-preserving rounding elsewhere.)
  - Total per core: DVE 5 passes * 8192 cols ~ 47 us; wire 16.78 MB in
    + 2.1 MB out ~ 53 us @ 358 GB/s -> wire-bound. ACT/PE/GPSIMD idle.
  - Loads per-plane on the sync HWDGE ring (first chunk's planes split
    in halves so the DVE chain starts ~1 plane early); int8 stores ride
    the scalar ring. bufs=4 on the x pool so slot recycling never
    throttles the load stream; small last chunk keeps the tail short.
"""

import numpy as np

import concourse.bacc as bacc
import concourse.mybir as mybir
import concourse.tile as tile
from concourse import bass_utils
from concourse import dve_ops as DO
from concourse.dve_spec import Spec, Src0, Src1, One, C0, lower, _has_src1
from concourse.dve_uop import DveOpSpec

T = 4
B_FULL = 64
C, H, W = 128, 32, 32
N_CORES = 8
B_LOC = B_FULL // N_CORES            # 8
N = B_LOC * C * H * W                # 1048576 elements per core per timestep
P = 128                              # SBUF partitions
NP = N // P                          # 8192 elements per partition row

_F32 = mybir.dt.float32
_I8 = mybir.dt.int8

# chunk widths (elements per partition); sum must be NP. Wire-bound, so
# DVE has slack; medium chunks amortize per-op overhead, small last
# chunk shortens the tail.
FS = [2048, 2048, 2048, 1536, 512]
assert sum(FS) == NP


def _register(name, spec):
    """Idempotently append a custom DveOp to the module registry, computing
    uops_sha at runtime (self-contained kernels can't check in pins)."""
    for o in DO.OPS:
        if o.name == name:
            return o
    row = DO._CUSTOM_DVE_ROW_BASE + len(DO.OPS)
    assert row < 0x20
    DO._SUB_OPCODE_FOR_NAME[name] = row
    shas = {}
    for ver in ("v3", "v4"):
        tmp = DveOpSpec(name=name, opcode=row, uops=lower(spec, ver=ver),
                        rd1_en=_has_src1(spec))
        shas[ver] = tmp.sha(ver)
    op = DO.DveOp(name, spec, subdim=False, uops_sha=shas)
    DO.OPS.append(op)
    DO.CUSTOM_DVE_SPECS[name] = spec
    return op


_s = Src0 * (Src0 <= One)
LIF_STEP = _register(
    "LIF_STEP_ANT",
    Spec(
        body=_s + Src1,
        reference=lambda in0, in1, c0, c1, c2: (
            in0 * (in0 <= 1.0) + in1
        ).astype(np.float32),
    ),
)
PACK2 = _register(
    "PACK_SPIKES2_ANT",
    Spec(
        body=(Src0 > One) * C0 + (Src1 > One),
        reference=lambda in0, in1, c0, c1, c2: (
            (in0 > 1.0).astype(np.float32) * c0 + (in1 > 1.0)
        ).astype(np.float32),
    ),
)

_nc_cache = None


def _build(fs=tuple(FS), bufs=4):
    nc = bacc.Bacc(
        "TRN2",
        target_bir_lowering=False,
        debug=False,
        enable_asserts=False,
    )
    x_d = nc.dram_tensor("x", [T, N], _F32, kind="ExternalInput").ap()
    y_d = nc.dram_tensor("y", [2, N], _I8, kind="ExternalOutput").ap()

    def xsl(t, base, f):
        return x_d[t, P * base : P * (base + f)].rearrange("(p f) -> p f", p=P)

    def ysl(i, base, f):
        return y_d[i, P * base : P * (base + f)].rearrange("(p f) -> p f", p=P)

    with tile.TileContext(nc) as tc:
        with (
            tc.tile_pool(name="xx", bufs=bufs) as xp,
            tc.tile_pool(name="yy", bufs=bufs) as yp,
        ):
            base = 0
            for j, F in enumerate(fs):
                sl = []
                for t in range(T):
                    xt = xp.tile([P, F], _F32, tag=f"x{t}", name=f"x{t}_{j}")
                    if j == 0 and t <= 1:
                        # split the first loads so the DVE chain starts
                        # after half a plane
                        h = F // 2
                        src = xsl(t, base, F)
                        nc.sync.dma_start(xt[:, :h], src[:, :h])
                        nc.sync.dma_start(xt[:, h:], src[:, h:])
                    else:
                        nc.sync.dma_start(xt[:], xsl(t, base, F))
                    sl.append(xt[:])

                v = nc.vector

                def pack_store(i, a, b, off=0, w=F, k=""):
                    y = yp.tile([P, w], _I8, tag=f"y{i}{k}", name=f"y{i}{k}_{j}")
                    v._custom_dve(PACK2, out=y[:], in0=a, in1=b, s0=2.0)
                    nc.scalar.dma_start(ysl(i, base, F)[:, off : off + w], y[:])

                if j == 0:
                    # halves: s1 = lif(s0, x1) then pack, per half
                    h = F // 2
                    for off in (0, h):
                        a, b = sl[0][:, off : off + h], sl[1][:, off : off + h]
                        v._custom_dve(LIF_STEP, out=b, in0=a, in1=b)
                    pack_store(0, sl[0], sl[1])
                else:
                    v._custom_dve(LIF_STEP, out=sl[1], in0=sl[0], in1=sl[1])
                    pack_store(0, sl[0], sl[1])
                v._custom_dve(LIF_STEP, out=sl[2], in0=sl[1], in1=sl[2])
                v._custom_dve(LIF_STEP, out=sl[3], in0=sl[2], in1=sl[3])
                pack_store(1, sl[2], sl[3])
                base += F

    nc.compile()
    return nc


def _get_nc():
    global _nc_cache
    if _nc_cache is None:
        _nc_cache = _build()
    return _nc_cache


def _run(x, **spmd_kwargs):
    x = np.asarray(x, dtype=np.float32)
    assert x.shape == (T, B_FULL, C, H, W), x.shape
    in_maps = [
        {
            "x": np.ascontiguousarray(
                x[:, c * B_LOC : (c + 1) * B_LOC]
            ).reshape(T, N)
        }
        for c in range(N_CORES)
    ]
    res = bass_utils.run_bass_kernel_spmd(
        _get_nc(), in_maps, core_ids=list(range(N_CORES)), **spmd_kwargs
    )
    out = np.empty((T, B_FULL, C, H, W), dtype=np.float32)
    sh = (B_LOC, C, H, W)
    for c in range(N_CORES):
        y = res.results[c]["y"]  # [2, N] int8, rows (t0,t1) and (t2,t3)
        b = slice(c * B_LOC, (c + 1) * B_LOC)
        out[0, b] = ((y[0] >> 1) & 1).astype(np.float32).reshape(sh)
        out[1, b] = (y[0] & 1).astype(np.float32).reshape(sh)
        out[2, b] = ((y[1] >> 1) & 1).astype(np.float32).reshape(sh)
        out[3, b] = (y[1] & 1).astype(np.float32).reshape(sh)
    return out, res


def kernel(x):
    out, _ = _run(x)
    return out


# revision 4
# speedup vs baseline: 1.1687x; 1.0471x over previous
"""LIF spike kernel (T=4 scan with threshold reset) on 8 TRN2 NeuronCores.

Recurrence per element (tau=1, thresh=1):
    s_t     = m_{t-1} + x_t
    spike_t = (s_t > 1)           -> output
    m_t     = s_t * (s_t <= 1)    -> threshold reset

Sharding: pure data-parallel over the batch axis (dim 1, 64 -> 8 per core).

Design (v2, custom-DVE):
  - Carry s_t (pre-reset membrane) instead of m_t: s_{t+1} = s_t*(s_t<=1)
    + x_{t+1} is ONE fused custom DVE instruction (LIF_STEP_ANT), so the
    serial recurrence costs 3 DVE passes instead of 6 (fp32 tensor ops
    run at 1 elem/lane/cycle regardless of fusion). Bit-exact: the mult
    is by an exact 0/1 mask and the add is a single IEEE fp32 add, same
    order as the reference.
  - Spikes leave the device packed 2-per-byte: y = (s_a>1)*2 + (s_b>1)
    (PACK_SPIKES2_ANT, int8 out straight from the DVE write port, values
    0..3 exact). Output wire traffic is 2 MiB/core vs 4 MiB for the int8
    sign planes and 16 MiB for f32. (s-1>0 <=> s>1 in fp32: Sterbenz on
    (1,2), sign-preserving rounding elsewhere.)
  - Total per core: DVE 5 passes * 8192 cols ~ 47 us; wire 16.78 MB in
    + 2.1 MB out ~ 53 us @ 358 GB/s -> wire-bound. ACT/PE/GPSIMD idle.
  - Loads per-plane on the sync HWDGE ring (first chunk's planes split
    in halves so the DVE chain starts ~1 plane early); int8 stores ride
    the scalar ring. bufs=4 on the x pool so slot recycling never
    throttles the load stream; small last chunk keeps the tail short.
"""

import numpy as np

import concourse.bacc as bacc
import concourse.mybir as mybir
import concourse.tile as tile
from concourse import bass_utils
from concourse import dve_ops as DO
from concourse.dve_spec import Spec, Src0, Src1, One, C0, lower, _has_src1
from concourse.dve_uop import DveOpSpec

T = 4
B_FULL = 64
C, H, W = 128, 32, 32
N_CORES = 8
B_LOC = B_FULL // N_CORES            # 8
N = B_LOC * C * H * W                # 1048576 elements per core per timestep
P = 128                              # SBUF partitions
NP = N // P                          # 8192 elements per partition row

_F32 = mybir.dt.float32
_I8 = mybir.dt.int8

# chunk widths (elements per partition); sum must be NP. Wire-bound, so
# DVE has slack; small first chunk lands fast (DVE starts early), big
# middle chunks amortize per-op + per-DMA overhead, small last chunks
# keep the post-load DVE/store tail short.
FS = [768, 2048, 2048, 2048, 1024, 256]
assert sum(FS) == NP


def _register(name, spec):
    """Idempotently append a custom DveOp to the module registry, computing
    uops_sha at runtime (self-contained kernels can't check in pins)."""
    for o in DO.OPS:
        if o.name == name:
            return o
    row = DO._CUSTOM_DVE_ROW_BASE + len(DO.OPS)
    assert row < 0x20
    DO._SUB_OPCODE_FOR_NAME[name] = row
    shas = {}
    for ver in ("v3", "v4"):
        tmp = DveOpSpec(name=name, opcode=row, uops=lower(spec, ver=ver),
                        rd1_en=_has_src1(spec))
        shas[ver] = tmp.sha(ver)
    op = DO.DveOp(name, spec, subdim=False, uops_sha=shas)
    DO.OPS.append(op)
    DO.CUSTOM_DVE_SPECS[name] = spec
    return op


_s = Src0 * (Src0 <= One)
LIF_STEP = _register(
    "LIF_STEP_ANT",
    Spec(
        body=_s + Src1,
        reference=lambda in0, in1, c0, c1, c2: (
            in0 * (in0 <= 1.0) + in1
        ).astype(np.float32),
    ),
)
PACK2 = _register(
    "PACK_SPIKES2_ANT",
    Spec(
        body=(Src0 > One) * C0 + (Src1 > One),
        reference=lambda in0, in1, c0, c1, c2: (
            (in0 > 1.0).astype(np.float32) * c0 + (in1 > 1.0)
        ).astype(np.float32),
    ),
)

_nc_cache = None


def _build(fs=tuple(FS), bufs=4):
    nc = bacc.Bacc(
        "TRN2",
        target_bir_lowering=False,
        debug=False,
        enable_asserts=False,
    )
    x_d = nc.dram_tensor("x", [T, N], _F32, kind="ExternalInput").ap()
    y_d = nc.dram_tensor("y", [2, N], _I8, kind="ExternalOutput").ap()

    def xsl(base, f):
        # [P, T, f] view of all four timestep planes for this chunk
        return x_d[:, P * base : P * (base + f)].rearrange(
            "t (p f) -> p t f", p=P
        )

    def ysl(base, f):
        return y_d[:, P * base : P * (base + f)].rearrange(
            "i (p f) -> p i f", p=P
        )

    with tile.TileContext(nc) as tc:
        with (
            tc.tile_pool(name="xx", bufs=bufs) as xp,
            tc.tile_pool(name="yy", bufs=bufs) as yp,
        ):
            base = 0
            for j, F in enumerate(fs):
                # one DMA for the whole chunk (4 planes): fewer
                # instructions + semaphores, bigger transfer
                xt = xp.tile([P, T, F], _F32, tag="x", name=f"x_{j}")
                nc.sync.dma_start(xt[:], xsl(base, F))
                sl = [xt[:, t] for t in range(T)]

                v = nc.vector
                y = yp.tile([P, 2, F], _I8, tag="y", name=f"y_{j}")
                v._custom_dve(LIF_STEP, out=sl[1], in0=sl[0], in1=sl[1])
                v._custom_dve(PACK2, out=y[:, 0], in0=sl[0], in1=sl[1], s0=2.0)
                v._custom_dve(LIF_STEP, out=sl[2], in0=sl[1], in1=sl[2])
                v._custom_dve(LIF_STEP, out=sl[3], in0=sl[2], in1=sl[3])
                v._custom_dve(PACK2, out=y[:, 1], in0=sl[2], in1=sl[3], s0=2.0)
                nc.scalar.dma_start(ysl(base, F), y[:])
                base += F

    nc.compile()
    return nc


def _get_nc():
    global _nc_cache
    if _nc_cache is None:
        _nc_cache = _build()
    return _nc_cache


def _run(x, **spmd_kwargs):
    x = np.asarray(x, dtype=np.float32)
    assert x.shape == (T, B_FULL, C, H, W), x.shape
    in_maps = [
        {
            "x": np.ascontiguousarray(
                x[:, c * B_LOC : (c + 1) * B_LOC]
            ).reshape(T, N)
        }
        for c in range(N_CORES)
    ]
    res = bass_utils.run_bass_kernel_spmd(
        _get_nc(), in_maps, core_ids=list(range(N_CORES)), **spmd_kwargs
    )
    out = np.empty((T, B_FULL, C, H, W), dtype=np.float32)
    sh = (B_LOC, C, H, W)
    for c in range(N_CORES):
        y = res.results[c]["y"]  # [2, N] int8, rows (t0,t1) and (t2,t3)
        b = slice(c * B_LOC, (c + 1) * B_LOC)
        out[0, b] = ((y[0] >> 1) & 1).astype(np.float32).reshape(sh)
        out[1, b] = (y[0] & 1).astype(np.float32).reshape(sh)
        out[2, b] = ((y[1] >> 1) & 1).astype(np.float32).reshape(sh)
        out[3, b] = (y[1] & 1).astype(np.float32).reshape(sh)
    return out, res


def kernel(x):
    out, _ = _run(x)
    return out


# revision 6
# speedup vs baseline: 1.2087x; 1.0343x over previous
"""LIF spike kernel (T=4 scan with threshold reset) on 8 TRN2 NeuronCores.

Recurrence per element (tau=1, thresh=1):
    s_t     = m_{t-1} + x_t
    spike_t = (s_t > 1)           -> output
    m_t     = s_t * (s_t <= 1)    -> threshold reset

Sharding: pure data-parallel over the batch axis (dim 1, 64 -> 8 per core).

Design (v2, custom-DVE):
  - Carry s_t (pre-reset membrane) instead of m_t: s_{t+1} = s_t*(s_t<=1)
    + x_{t+1} is ONE fused custom DVE instruction (LIF_STEP_ANT), so the
    serial recurrence costs 3 DVE passes instead of 6 (fp32 tensor ops
    run at 1 elem/lane/cycle regardless of fusion). Bit-exact: the mult
    is by an exact 0/1 mask and the add is a single IEEE fp32 add, same
    order as the reference.
  - Spikes leave the device packed 2-per-byte: y = (s_a>1)*2 + (s_b>1)
    (PACK_SPIKES2_ANT, int8 out straight from the DVE write port, values
    0..3 exact). Output wire traffic is 2 MiB/core vs 4 MiB for the int8
    sign planes and 16 MiB for f32. (s-1>0 <=> s>1 in fp32: Sterbenz on
    (1,2), sign-preserving rounding elsewhere.)
  - Total per core: DVE 5 passes * 8192 cols ~ 47 us; wire 16.78 MB in
    + 2.1 MB out ~ 53 us @ 358 GB/s -> wire-bound. ACT/PE/GPSIMD idle.
  - Loads per-plane on the sync HWDGE ring (first chunk's planes split
    in halves so the DVE chain starts ~1 plane early); int8 stores ride
    the scalar ring. bufs=4 on the x pool so slot recycling never
    throttles the load stream; small last chunk keeps the tail short.
"""

import numpy as np

import concourse.bacc as bacc
import concourse.mybir as mybir
import concourse.tile as tile
from concourse import bass_utils
from concourse import dve_ops as DO
from concourse.dve_spec import Spec, Src0, Src1, One, C0, lower, _has_src1
from concourse.dve_uop import DveOpSpec

T = 4
B_FULL = 64
C, H, W = 128, 32, 32
N_CORES = 8
B_LOC = B_FULL // N_CORES            # 8
N = B_LOC * C * H * W                # 1048576 elements per core per timestep
P = 128                              # SBUF partitions
NP = N // P                          # 8192 elements per partition row

_F32 = mybir.dt.float32
_I8 = mybir.dt.int8

# chunk widths (elements per partition); sum must be NP. Wire-bound, so
# DVE has slack; small first chunk lands fast (DVE starts early), big
# middle chunks amortize per-op + per-DMA overhead, small last chunks
# keep the post-load DVE/store tail short.
FS = [256, 1024, 2048, 2048, 1792, 1024]
assert sum(FS) == NP


def _register(name, spec):
    """Idempotently append a custom DveOp to the module registry, computing
    uops_sha at runtime (self-contained kernels can't check in pins)."""
    for o in DO.OPS:
        if o.name == name:
            return o
    row = DO._CUSTOM_DVE_ROW_BASE + len(DO.OPS)
    assert row < 0x20
    DO._SUB_OPCODE_FOR_NAME[name] = row
    shas = {}
    for ver in ("v3", "v4"):
        tmp = DveOpSpec(name=name, opcode=row, uops=lower(spec, ver=ver),
                        rd1_en=_has_src1(spec))
        shas[ver] = tmp.sha(ver)
    op = DO.DveOp(name, spec, subdim=False, uops_sha=shas)
    DO.OPS.append(op)
    DO.CUSTOM_DVE_SPECS[name] = spec
    return op


_s = Src0 * (Src0 <= One)
LIF_STEP = _register(
    "LIF_STEP_ANT",
    Spec(
        body=_s + Src1,
        reference=lambda in0, in1, c0, c1, c2: (
            in0 * (in0 <= 1.0) + in1
        ).astype(np.float32),
    ),
)
PACK2 = _register(
    "PACK_SPIKES2_ANT",
    Spec(
        body=(Src0 > One) * C0 + (Src1 > One),
        reference=lambda in0, in1, c0, c1, c2: (
            (in0 > 1.0).astype(np.float32) * c0 + (in1 > 1.0)
        ).astype(np.float32),
    ),
)

_nc_cache = None


def _build(fs=tuple(FS), bufs=4):
    nc = bacc.Bacc(
        "TRN2",
        target_bir_lowering=False,
        debug=False,
        enable_asserts=False,
    )
    x_d = nc.dram_tensor("x", [T, N], _F32, kind="ExternalInput").ap()
    y_d = nc.dram_tensor("y", [2, N], _I8, kind="ExternalOutput").ap()

    def xsl(base, f):
        # [P, T, f] view of all four timestep planes for this chunk
        return x_d[:, P * base : P * (base + f)].rearrange(
            "t (p f) -> p t f", p=P
        )

    def ysl(base, f):
        return y_d[:, P * base : P * (base + f)].rearrange(
            "i (p f) -> p i f", p=P
        )

    with tile.TileContext(nc) as tc:
        with (
            tc.tile_pool(name="xx", bufs=bufs) as xp,
            tc.tile_pool(name="yy", bufs=bufs) as yp,
        ):
            base = 0
            for j, F in enumerate(fs):
                # two DMAs per chunk (planes {0,1} and {2,3}): large
                # transfers for wire rate, but the DVE chain can start
                # after the first half lands
                xt = xp.tile([P, T, F], _F32, tag="x", name=f"x_{j}")
                src = xsl(base, F)
                nc.sync.dma_start(xt[:, 0:2], src[:, 0:2])
                nc.sync.dma_start(xt[:, 2:4], src[:, 2:4])
                sl = [xt[:, t] for t in range(T)]

                v = nc.vector
                y = yp.tile([P, 2, F], _I8, tag="y", name=f"y_{j}")
                v._custom_dve(LIF_STEP, out=sl[1], in0=sl[0], in1=sl[1])
                v._custom_dve(PACK2, out=y[:, 0], in0=sl[0], in1=sl[1], s0=2.0)
                v._custom_dve(LIF_STEP, out=sl[2], in0=sl[1], in1=sl[2])
                v._custom_dve(LIF_STEP, out=sl[3], in0=sl[2], in1=sl[3])
                v._custom_dve(PACK2, out=y[:, 1], in0=sl[2], in1=sl[3], s0=2.0)
                nc.scalar.dma_start(ysl(base, F), y[:])
                base += F

    nc.compile()
    return nc


def _get_nc():
    global _nc_cache
    if _nc_cache is None:
        _nc_cache = _build()
    return _nc_cache


def _run(x, **spmd_kwargs):
    x = np.asarray(x, dtype=np.float32)
    assert x.shape == (T, B_FULL, C, H, W), x.shape
    in_maps = [
        {
            "x": np.ascontiguousarray(
                x[:, c * B_LOC : (c + 1) * B_LOC]
            ).reshape(T, N)
        }
        for c in range(N_CORES)
    ]
    res = bass_utils.run_bass_kernel_spmd(
        _get_nc(), in_maps, core_ids=list(range(N_CORES)), **spmd_kwargs
    )
    out = np.empty((T, B_FULL, C, H, W), dtype=np.float32)
    sh = (B_LOC, C, H, W)
    for c in range(N_CORES):
        y = res.results[c]["y"]  # [2, N] int8, rows (t0,t1) and (t2,t3)
        b = slice(c * B_LOC, (c + 1) * B_LOC)
        out[0, b] = ((y[0] >> 1) & 1).astype(np.float32).reshape(sh)
        out[1, b] = (y[0] & 1).astype(np.float32).reshape(sh)
        out[2, b] = ((y[1] >> 1) & 1).astype(np.float32).reshape(sh)
        out[3, b] = (y[1] & 1).astype(np.float32).reshape(sh)
    return out, res


def kernel(x):
    out, _ = _run(x)
    return out
